# revision 5
# baseline (speedup 1.0000x reference)
"""Two-layer GAT on 8 Trainium2 NeuronCores (Bass/Tile) — pair-gather design.

Host: degree-sort nodes desc (self-loops handled on-device), pad to VPAD
(multiple of 8*128), assign round-robin at 128-node blocks to 8 cores
(sorted-rank s -> block g=s//128, lane=s%128 -> core g%8, local block j=g//8).
Per block-rank j the chunk count K[j] = max non-self in-degree over the 1024
nodes of that rank (shared SPMD schedule).  Each non-self edge occupies one
slot (sorted by src row for locality); chunk k = slot k of all 128 lanes.

The gather tables hold bf16 node rows ([h] 256B for L1, [h2] 128B for L2);
one dma_gather descriptor fetches the PAIR of adjacent rows (2t, 2t+1) with
idx = src_row>>1 — fits int16 with NO address split, eliminating the lo/hi
slot-padding of the max(lo)+max(hi) schedule.  The even/odd half is selected
by host-precomputed masks sv_e/sv_o (valid*parity) folded into the V build:
V = ge*(w_e*sv_e) + go*(w_o*sv_o), w = exp(prelu(asrc+adst)).  asrc per edge
comes from DVE dots of the gathered halves with a_src; adst / self-loop attn
logits come fp32 from the augmented h-phase matmul W_aug = [W|W@a_src|W@a_dst].
Self-loop contributions are built from local h rows (no gather) as the
accumulation-opening matmul of each block.

AllGathers replicate the packed bf16 tables (Shared outputs for the fast
HBM-HBM path).  Epilogue fuses normalization + the next layer's h-phase.
"""

import numpy as np

NCORES = 8
F_IN = 128
HID = 64
HEADS = 2
OUT = 64
NEG_SLOPE = 0.2

V1C = 130    # L1 V cols: [w0*h0 | w1*h1 | w0 | w1]
V2C = 65     # L2 V cols: [w*h2 | w]
GBATCH = 32  # max chunks per dma_gather

TRACE = False
TRACE_DIR = None
_cache = {}


def _build_schedule(edge_index, n_nodes):
    ei = np.asarray(edge_index).astype(np.int64)
    src = ei[0]
    dst = ei[1]
    deg = np.bincount(dst, minlength=n_nodes)  # non-self degree

    stripe = NCORES * 128
    vpad = ((n_nodes + stripe - 1) // stripe) * stripe
    pc = vpad // NCORES
    nb = pc // 128

    degp = np.zeros(vpad, np.int64)
    degp[:n_nodes] = deg
    order = np.argsort(-degp, kind="stable")
    rank = np.empty(vpad, np.int64)
    rank[order] = np.arange(vpad)

    s = np.arange(vpad)
    g = s // 128
    lane = s % 128
    row_of_rank = (g % NCORES) * pc + (g // NCORES) * 128 + lane
    row_of_node = row_of_rank[rank[:n_nodes]]

    e_dstrow = row_of_node[dst]
    e_srcrow = row_of_node[src]

    cnt = np.bincount(e_dstrow, minlength=vpad)

    # shared chunk count per block-rank j: max over the 8 cores' j-th blocks
    jj = (np.arange(vpad) % pc) // 128
    K = np.zeros(nb, np.int64)
    np.maximum.at(K, jj, cnt)
    nch = int(K.sum())
    chunk_base = np.concatenate([[0], np.cumsum(K)])[:-1]

    # slot assignment: edges of a dst grouped, sorted by src row (locality)
    key = e_dstrow * vpad + e_srcrow
    ord_e = np.argsort(key, kind="stable")
    ds = e_dstrow[ord_e]
    first = np.r_[True, ds[1:] != ds[:-1]]
    grp_start = np.flatnonzero(first)
    grp_id = np.cumsum(first) - 1
    slot = np.arange(ds.shape[0]) - grp_start[grp_id]
    c = ds // pc
    j = (ds % pc) // 128
    ln = ds % 128
    assert (slot < K[j]).all()
    pos = chunk_base[j] + slot

    srcs = e_srcrow[ord_e]
    idx_stream = np.zeros((NCORES, 128, nch), np.int16)
    sv_e = np.zeros((NCORES, 128, nch), np.float32)
    sv_o = np.zeros((NCORES, 128, nch), np.float32)
    idx_stream[c, ln, pos] = (srcs >> 1).astype(np.int16)
    sv_e[c, ln, pos] = (srcs % 2 == 0).astype(np.float32)
    sv_o[c, ln, pos] = (srcs % 2 == 1).astype(np.float32)

    # wrapped int16 layout for dma_gather: chunk k -> columns 8k:8k+8 of
    # [128, 8*nch]; within a chunk flat[i] -> [i % 16, i // 16], replicated
    # over the 8 16-partition groups.
    iw = idx_stream.transpose(0, 2, 1).reshape(NCORES, nch, 8, 16)
    iw = iw.transpose(0, 3, 1, 2).reshape(NCORES, 16, nch * 8)
    idx_wrapped = np.tile(iw, (1, 8, 1))

    return dict(vpad=vpad, pc=pc, nb=nb, K=K, nch=nch, chunk_base=chunk_base,
                row_of_node=row_of_node,
                idx_wrapped=np.ascontiguousarray(idx_wrapped),
                sv_e=sv_e, sv_o=sv_o)


def _build_program(vpad, pc, nb, K, nch, chunk_base):
    import concourse.bacc as bacc
    import concourse.mybir as mybir
    import concourse.tile as tile
    from concourse.masks import make_identity

    F32 = mybir.dt.float32
    BF16 = mybir.dt.bfloat16
    I16 = mybir.dt.int16
    ACTF = mybir.ActivationFunctionType
    ALU = mybir.AluOpType

    nc = bacc.Bacc("TRN2", target_bir_lowering=False, debug=False,
                   num_devices=NCORES)

    xt_d = nc.dram_tensor("xt", [128, pc], F32, kind="ExternalInput")
    idx_d = nc.dram_tensor("idx", [128, nch * 8], I16, kind="ExternalInput")
    sve_d = nc.dram_tensor("sve", [128, nch], F32, kind="ExternalInput")
    svo_d = nc.dram_tensor("svo", [128, nch], F32, kind="ExternalInput")
    w1_d = nc.dram_tensor("w1aug", [128, 132], F32, kind="ExternalInput")
    w2_d = nc.dram_tensor("w2aug", [128, 66], BF16, kind="ExternalInput")
    b1_d = nc.dram_tensor("b1rep", [128, 128], F32, kind="ExternalInput")
    b2_d = nc.dram_tensor("b2rep", [128, 64], F32, kind="ExternalInput")
    as1_d = nc.dram_tensor("asrc1rep", [128, 128], BF16, kind="ExternalInput")
    as2_d = nc.dram_tensor("asrc2rep", [128, 64], BF16, kind="ExternalInput")
    out_d = nc.dram_tensor("out", [pc, OUT], F32, kind="ExternalOutput")

    # flat batches of <= GBATCH chunks, spanning at most 2 blocks each
    blk_of = np.repeat(np.arange(nb), K)
    last_idx = (chunk_base + K - 1)
    last_of = np.zeros(max(nch, 1), bool)
    if (K > 0).any():
        last_of[last_idx[K > 0]] = True
    batches = []
    done = 0
    while done < nch:
        j0 = int(blk_of[done])
        # end of block j0+1 (or j0 if it's the last block with chunks)
        lim = done + GBATCH
        jend = min(j0 + 2, nb)
        blk_lim = int(chunk_base[jend - 1] + K[jend - 1]) if jend > 0 else 0
        gl = min(lim, blk_lim, nch) - done
        assert gl > 0
        batches.append((done, gl))
        done += gl

    with tile.TileContext(nc) as tc:
        with (
            tc.tile_pool(name="const", bufs=1) as cp,
            tc.tile_pool(name="dram", bufs=1, space="DRAM") as dp,
            tc.tile_pool(name="xs", bufs=3) as xp,
            tc.tile_pool(name="psh", bufs=2, space="PSUM") as psh,
            tc.tile_pool(name="g", bufs=3) as gp,
            tc.tile_pool(name="tmp", bufs=3) as tp,
            tc.tile_pool(name="v", bufs=2) as vp,
            tc.tile_pool(name="wz", bufs=6) as wp,
            tc.tile_pool(name="psa", bufs=4, space="PSUM") as psa,
            tc.tile_pool(name="pst", bufs=1, space="PSUM") as pst,
            tc.tile_pool(name="epi", bufs=3) as ep,
        ):
            ident = cp.tile([128, 128], BF16)
            make_identity(nc, ident[:])
            identf = cp.tile([128, 128], F32)
            make_identity(nc, identf[:])
            w1_sb = cp.tile([128, 132], F32)
            w2_sb = cp.tile([128, 66], BF16)
            b1_sb = cp.tile([128, 128], F32)
            b2_sb = cp.tile([128, 64], F32)
            as1_sb = cp.tile([128, 128], BF16)
            as2_sb = cp.tile([128, 64], BF16)
            idx_t = cp.tile([128, nch * 8], I16)
            sve_t = cp.tile([128, nch], F32)
            svo_t = cp.tile([128, nch], F32)
            for t, d in ((w1_sb, w1_d), (w2_sb, w2_d), (b1_sb, b1_d),
                         (b2_sb, b2_d), (as1_sb, as1_d), (as2_sb, as2_d),
                         (idx_t, idx_d), (sve_t, sve_d), (svo_t, svo_d)):
                nc.sync.dma_start(out=t[:], in_=d[:])

            # local h rows (bf16) and per-node attn logits, SBUF-resident
            hloc = cp.tile([128, nb * 128], BF16)
            h2loc = cp.tile([128, nb * 64], BF16)
            a1_sb = cp.tile([128, nb * 4], F32)   # [asrc0,asrc1,adst0,adst1]
            a2_sb = cp.tile([128, nb * 2], F32)   # [asrc2, adst2]

            t1loc = dp.tile([pc, 128], BF16)
            t1full = dp.tile([vpad, 128], BF16, addr_space="Shared")
            t2loc = dp.tile([pc, 64], BF16)
            t2full = dp.tile([vpad, 64], BF16, addr_space="Shared")

            # ---- Phase 1: L1 h-phase (single bulk x^T load) ----
            xt_all = cp.tile([128, pc], F32, name="xt_all")
            nc.sync.dma_start(out=xt_all[:], in_=xt_d[:])
            for j in range(nb):
                ps = psh.tile([128, 132], F32, tag="psh")
                nc.tensor.matmul(ps[:],
                                 lhsT=xt_all[:, j * 128:(j + 1) * 128],
                                 rhs=w1_sb[:], start=True, stop=True)
                nc.vector.tensor_tensor(out=hloc[:, j * 128:(j + 1) * 128],
                                        in0=ps[:, 0:128], in1=b1_sb[:],
                                        op=ALU.add)
                nc.scalar.activation(a1_sb[:, j * 4:j * 4 + 4],
                                     ps[:, 128:132], ACTF.Copy)
                nc.sync.dma_start(out=t1loc[j * 128:(j + 1) * 128, :],
                                  in_=hloc[:, j * 128:(j + 1) * 128])

            # ---- Phase 2: AllGather L1 table ----
            nc.gpsimd.collective_compute(
                "AllGather", mybir.AluOpType.bypass,
                replica_groups=[list(range(NCORES))],
                ins=[t1loc[:]], outs=[t1full[:]],
            )

            def agg_layer(layer):
                if layer == 1:
                    tab = t1full[:].rearrange("(t two) c -> t (two c)", two=2)
                    hl, asb, al, vcols, heads, grow = (
                        hloc, as1_sb, a1_sb, V1C, 2, 256)
                else:
                    tab = t2full[:].rearrange("(t two) c -> t (two c)", two=2)
                    hl, asb, al, vcols, heads, grow = (
                        h2loc, as2_sb, a2_sb, V2C, 1, 128)
                hdim = (vcols - heads) // heads  # 64
                hrow = grow // 2

                psums = {}

                def start_block(j):
                    # self-loop chunk from local h rows + fp32 attn logits
                    psum = psa.tile([128, vcols], F32, tag="psa")
                    psums[j] = psum
                    zs = wp.tile([128, heads], F32, tag="zs")
                    if layer == 1:
                        av = al[:, j * 4:j * 4 + 2]
                        bv = al[:, j * 4 + 2:j * 4 + 4]
                    else:
                        av = al[:, j * 2:j * 2 + 1]
                        bv = al[:, j * 2 + 1:j * 2 + 2]
                    nc.vector.tensor_tensor(out=zs[:], in0=av, in1=bv,
                                            op=ALU.add)
                    nc.scalar.activation(zs[:], zs[:], ACTF.Prelu,
                                         alpha=NEG_SLOPE)
                    nc.scalar.activation(zs[:], zs[:], ACTF.Exp)
                    zsb = wp.tile([128, heads], BF16, tag="zsb")
                    nc.scalar.activation(zsb[:], zs[:], ACTF.Copy)
                    vs = wp.tile([128, vcols], BF16, tag="vs")
                    for h in range(heads):
                        nc.vector.tensor_tensor(
                            out=vs[:, h * hdim:(h + 1) * hdim],
                            in0=hl[:, j * hrow + h * hdim:
                                   j * hrow + (h + 1) * hdim],
                            in1=zsb[:, h:h + 1].to_broadcast([128, hdim]),
                            op=ALU.mult)
                    nc.scalar.activation(vs[:, heads * hdim:vcols], zs[:],
                                         ACTF.Copy)
                    nc.tensor.matmul(psum[:], lhsT=ident[:], rhs=vs[:],
                                     start=True, stop=(int(K[j]) == 0))

                def finish_block(j):
                    psum = psums.pop(j)
                    dsafe = wp.tile([128, heads], F32, tag="dsafe")
                    nc.vector.tensor_scalar_add(
                        dsafe[:], psum[:, heads * hdim:vcols], 1e-30)
                    rden = wp.tile([128, heads], F32, tag="rden")
                    nc.vector.reciprocal(rden[:], dsafe[:])
                    if layer == 1:
                        h2pre = ep.tile([128, 128], F32, tag="h2pre")
                        for h in range(heads):
                            nc.scalar.activation(
                                h2pre[:, h * hdim:(h + 1) * hdim],
                                psum[:, h * hdim:(h + 1) * hdim],
                                ACTF.Relu, scale=rden[:, h:h + 1])
                        tps = pst.tile([128, 128], F32, tag="tps")
                        nc.tensor.transpose(out=tps[:], in_=h2pre[:],
                                            identity=identf[:])
                        h2t = ep.tile([128, 128], BF16, tag="h2t")
                        nc.scalar.activation(h2t[:], tps[:], ACTF.Copy)
                        ps3 = psh.tile([128, 66], F32, tag="psh")
                        nc.tensor.matmul(ps3[:], lhsT=h2t[:], rhs=w2_sb[:],
                                         start=True, stop=True)
                        nc.vector.tensor_tensor(
                            out=h2loc[:, j * 64:(j + 1) * 64],
                            in0=ps3[:, 0:64], in1=b2_sb[:], op=ALU.add)
                        nc.scalar.activation(a2_sb[:, j * 2:j * 2 + 2],
                                             ps3[:, 64:66], ACTF.Copy)
                        nc.sync.dma_start(out=t2loc[j * 128:(j + 1) * 128, :],
                                          in_=h2loc[:, j * 64:(j + 1) * 64])
                    else:
                        ob = ep.tile([128, OUT], F32, tag="ob")
                        nc.scalar.activation(ob[:], psum[:, 0:OUT],
                                             ACTF.Sigmoid, scale=rden[:, 0:1])
                        nc.sync.dma_start(out=out_d[j * 128:(j + 1) * 128, :],
                                          in_=ob[:])

                def open_blocks_from(j):
                    while j < nb:
                        start_block(j)
                        if int(K[j]) > 0:
                            return
                        finish_block(j)
                        j += 1

                open_blocks_from(0)
                for (b0, gl) in batches:
                    gt = gp.tile([128, GBATCH * grow], BF16, tag="g")
                    gv = gt[:, 0:gl * grow].rearrange("p (k c) -> p k c",
                                                      c=grow)
                    nc.gpsimd.dma_gather(
                        gv, tab,
                        idx_t[:, b0 * 8:(b0 + gl) * 8],
                        gl * 128, gl * 128, grow,
                        single_packet=False, queue_num=0,
                    )
                    # dots de/do [128, gl, heads] = <g_half, a_src> per head
                    dds = []
                    for half, tag in ((0, "de"), (1, "do")):
                        tmv = tp.tile([128, GBATCH * hrow], BF16, tag="tm")
                        tvv = tmv[:, 0:gl * hrow].rearrange(
                            "p (k c) -> p k c", c=hrow)
                        nc.vector.tensor_tensor(
                            out=tvv,
                            in0=gv[:, :, half * hrow:(half + 1) * hrow],
                            in1=asb[:, 0:hrow].rearrange(
                                "p (o c) -> p o c", o=1).to_broadcast(
                                [128, gl, hrow]),
                            op=ALU.mult)
                        dd = wp.tile([128, GBATCH * 2], F32, tag=tag)
                        nc.vector.tensor_reduce(
                            out=dd[:, 0:gl * heads].rearrange(
                                "p (k h) -> p k h", h=heads),
                            in_=tmv[:, 0:gl * hrow].rearrange(
                                "p (k h c) -> p k h c", h=heads, c=hdim),
                            axis=mybir.AxisListType.X, op=ALU.add)
                        dds.append(dd)
                    # w_e/w_o = exp(prelu(d + adst)) * sv_{e,o}
                    wes = []
                    for dd, svt in ((dds[0], sve_t), (dds[1], svo_t)):
                        ddv = dd[:, 0:gl * heads].rearrange(
                            "p (k h) -> p k h", h=heads)
                        c0 = b0
                        while c0 < b0 + gl:
                            j = int(blk_of[c0])
                            c1 = min(b0 + gl, int(chunk_base[j] + K[j]))
                            for h in range(heads):
                                if layer == 1:
                                    bias = al[:, j * 4 + 2 + h:j * 4 + 3 + h]
                                else:
                                    bias = al[:, j * 2 + 1:j * 2 + 2]
                                nc.scalar.activation(
                                    ddv[:, c0 - b0:c1 - b0, h],
                                    ddv[:, c0 - b0:c1 - b0, h],
                                    ACTF.Prelu, bias=bias, alpha=NEG_SLOPE)
                            c0 = c1
                        nc.scalar.activation(ddv[:, :, :], ddv[:, :, :],
                                             ACTF.Exp)
                        we = wp.tile([128, GBATCH * 2], BF16,
                                     tag="we" if svt is sve_t else "wo")
                        wev = we[:, 0:gl * heads].rearrange(
                            "p (k h) -> p k h", h=heads)
                        nc.vector.tensor_tensor(
                            out=wev, in0=ddv,
                            in1=svt[:, b0:b0 + gl].to_broadcast(
                                [128, gl, heads]),
                            op=ALU.mult)
                        wes.append(wev)
                    wev, wov = wes
                    # V build: [sum_h w*h | w], w = we + wo (selected half)
                    vv = vp.tile([128, GBATCH * vcols], BF16, tag="v")
                    vvv = vv[:, 0:gl * vcols].rearrange("p (k c) -> p k c",
                                                        c=vcols)
                    tmb = tp.tile([128, GBATCH * hrow], BF16, tag="tm")
                    tbv = tmb[:, 0:gl * hrow].rearrange("p (k c) -> p k c",
                                                        c=hrow)
                    for h in range(heads):
                        nc.vector.tensor_tensor(
                            out=vvv[:, :, h * hdim:(h + 1) * hdim],
                            in0=gv[:, :, h * hdim:(h + 1) * hdim],
                            in1=wev[:, :, h:h + 1].to_broadcast(
                                [128, gl, hdim]),
                            op=ALU.mult)
                        nc.vector.tensor_tensor(
                            out=tbv[:, :, h * hdim:(h + 1) * hdim],
                            in0=gv[:, :, hrow + h * hdim:
                                   hrow + (h + 1) * hdim],
                            in1=wov[:, :, h:h + 1].to_broadcast(
                                [128, gl, hdim]),
                            op=ALU.mult)
                    nc.vector.tensor_tensor(
                        out=vvv[:, :, 0:heads * hdim],
                        in0=vvv[:, :, 0:heads * hdim],
                        in1=tbv[:, :, 0:heads * hdim], op=ALU.add)
                    nc.vector.tensor_tensor(
                        out=vvv[:, :, heads * hdim:vcols],
                        in0=wev, in1=wov, op=ALU.add)
                    # accumulate per chunk; close/open blocks as they end
                    for cix in range(b0, b0 + gl):
                        j = int(blk_of[cix])
                        nc.tensor.matmul(
                            psums[j][:], lhsT=ident[:],
                            rhs=vv[:, (cix - b0) * vcols:
                                   (cix - b0 + 1) * vcols],
                            start=False, stop=bool(last_of[cix]))
                        if last_of[cix]:
                            finish_block(j)
                            open_blocks_from(j + 1)
                assert not psums, f"unfinished blocks: {list(psums)}"
                if nch == 0:
                    pass

            agg_layer(1)
            nc.gpsimd.collective_compute(
                "AllGather", mybir.AluOpType.bypass,
                replica_groups=[list(range(NCORES))],
                ins=[t2loc[:]], outs=[t2full[:]],
            )
            agg_layer(2)

    nc.finalize()
    return nc


def kernel(x, edge_index, W1, att_src1, att_dst1, b1, W2, att_src2, att_dst2,
           b2):
    import ml_dtypes
    from concourse import bass_utils

    x = np.asarray(x, np.float32)
    W1 = np.asarray(W1, np.float32)
    W2 = np.asarray(W2, np.float32)
    att_src1 = np.asarray(att_src1, np.float32)
    att_dst1 = np.asarray(att_dst1, np.float32)
    att_src2 = np.asarray(att_src2, np.float32)
    att_dst2 = np.asarray(att_dst2, np.float32)
    b1 = np.asarray(b1, np.float32)
    b2 = np.asarray(b2, np.float32)
    n_nodes = x.shape[0]

    sch = _build_schedule(edge_index, n_nodes)
    vpad, pc = sch["vpad"], sch["pc"]

    W1r = W1.reshape(F_IN, HEADS, HID)
    w1_aug = np.zeros((F_IN, 132), np.float32)
    w1_aug[:, 0:HEADS * HID] = W1
    for h in range(HEADS):
        w1_aug[:, HEADS * HID + h] = W1r[:, h, :] @ att_src1[h]
        w1_aug[:, HEADS * HID + HEADS + h] = W1r[:, h, :] @ att_dst1[h]
    w2_aug = np.zeros((HEADS * HID, 66), np.float32)
    w2_aug[:, 0:OUT] = W2
    w2_aug[:, OUT] = W2 @ att_src2[0]
    w2_aug[:, OUT + 1] = W2 @ att_dst2[0]
    b1_rep = np.broadcast_to(b1, (128, HEADS * HID)).copy()
    b2_rep = np.broadcast_to(b2, (128, OUT)).copy()
    asrc1_rep = np.broadcast_to(att_src1.reshape(-1), (128, 128)).astype(
        ml_dtypes.bfloat16)
    asrc2_rep = np.broadcast_to(att_src2[0], (128, 64)).astype(
        ml_dtypes.bfloat16)

    x_rho = np.zeros((vpad, F_IN), np.float32)
    x_rho[sch["row_of_node"]] = x

    key = (vpad, sch["nch"], tuple(sch["K"].tolist()))
    if key not in _cache:
        _cache[key] = _build_program(vpad, pc, sch["nb"], sch["K"],
                                     sch["nch"], sch["chunk_base"])
    nc = _cache[key]

    in_maps = []
    for c in range(NCORES):
        in_maps.append({
            "xt": np.ascontiguousarray(x_rho[c * pc:(c + 1) * pc].T),
            "idx": sch["idx_wrapped"][c],
            "sve": sch["sv_e"][c],
            "svo": sch["sv_o"][c],
            "w1aug": w1_aug,
            "w2aug": w2_aug.astype(ml_dtypes.bfloat16),
            "b1rep": b1_rep,
            "b2rep": b2_rep,
            "asrc1rep": asrc1_rep,
            "asrc2rep": asrc2_rep,
        })
    res = bass_utils.run_bass_kernel_spmd(nc, in_maps,
                                          core_ids=list(range(NCORES)),
                                          trace=TRACE,
                                          tmpdir=TRACE_DIR if TRACE else None)
    kernel.last_exec_ns = res.exec_time_ns
    kernel.last_mean_ns = res.mean_exec_time_ns
    out_all = np.concatenate([res.results[c]["out"] for c in range(NCORES)], 0)
    return out_all[sch["row_of_node"][:n_nodes]]
